# revision 37
# baseline (speedup 1.0000x reference)
"""Self-contained Trainium2 Bass kernel for the 3-layer GCN
(nn_Decoder_64020782514981): kernel(**inputs) -> np.ndarray [20000, 128] f32.

Design (v3):

- Nodes are assigned to (core, block) bins by host-side load balancing
  (LPT on in-degree) so every 125-node block has <= 2048 in-edges; the
  node->bin permutation is inverted on the host when unsharding.

- Layer 1 messages (x[src] rows in edge-slot order) are pre-gathered on
  the host (pure index plumbing) and streamed per block with sequential
  HWDGE DMA — no SWDGE descriptor-generation cost for layer 1.

- h1 / z3 tables are distributed with chunked AllGather ([7,7,5,1]
  blocks per chunk).  Layer 2/3 gathers are split per chunk-GROUP
  ([[0],[1],[2,3]]) with the source AP sliced to the group's row range,
  so each sub-gather's Tile dependency is only that group's AllGathers:
  the GpSimd descriptor generation for layer 2 starts mid-phase-A and
  layer 3's starts mid-phase-B, hiding the Q7 serial cost.

- Self-loop terms are folded into the TensorE PSUM accumulation as a
  diag(dinv^2) matmul per block (no DVE in the block pipeline).

- Gather index streams are padded with -1 (skipped by the DGE) with the
  exact valid count (bucketed to /64) passed in a register.

- Weight/S streams ride the scalar HWDGE ring; per-block message streams
  ride the sync ring, so the two SDMA descriptor paths run in parallel.
"""
import numpy as np
import ml_dtypes

from concourse import bass, bacc, mybir
import concourse.tile as tile

P = 128
F32 = mybir.dt.float32
BF16 = mybir.dt.bfloat16
I16 = mybir.dt.int16

N = 20000
E_TOTAL = 320000
HID = 256
OUT = 128
NC = 8
SH = N // NC            # 2500 nodes per core
BLK = 125               # dst rows per block
NT = SH // BLK          # 20 blocks per core
KC = HID // P           # 2 feature chunks
CPB_CAP = 16            # max edge chunks per block the balancer guarantees
CHUNK_BLOCKS = [7, 7, 5, 1]   # AllGather chunk sizes in blocks
NCH = len(CHUNK_BLOCKS)
GROUPS = [[0], [1], [2, 3]]   # gather split by chunk groups


class Cfg:
    def __init__(self, has_bias, transport="ag"):
        self.N, self.E, self.HID, self.OUT = N, E_TOTAL, HID, OUT
        self.NC = NC
        self.SH = SH
        self.BLK = BLK
        self.NT = NT
        self.KC = KC
        self.has_bias = has_bias
        self.transport = transport


def _balance_nodes(dst):
    """Assign nodes to 160 bins of exactly BLK nodes, balancing in-edge
    counts (LPT greedy).  Returns perm[newid] = oldid in bin order."""
    nbins = NC * NT
    deg = np.bincount(dst, minlength=N)
    order = np.argsort(-deg, kind="stable")
    bin_load = np.zeros(nbins, np.int64)
    bin_cnt = np.zeros(nbins, np.int32)
    bin_members = [[] for _ in range(nbins)]
    import heapq
    heap = [(0, b) for b in range(nbins)]
    heapq.heapify(heap)
    for node in order:
        while True:
            load, b = heapq.heappop(heap)
            if bin_cnt[b] < BLK and load == bin_load[b]:
                break
        bin_members[b].append(node)
        bin_cnt[b] += 1
        bin_load[b] += deg[node]
        if bin_cnt[b] < BLK:
            heapq.heappush(heap, (bin_load[b], b))
    perm = np.concatenate([np.asarray(m, np.int64) for m in bin_members])
    assert perm.shape[0] == N
    return perm, int(bin_load.max())


def _wrap_idxs(I):
    # [k*128] int -> [128, k*8] wrapped-16 layout
    w16 = I.reshape(-1, 16).T
    return np.tile(w16, (8, 1)).astype(np.int16)


def prep(x, edge_index, W1, b1, W2, b2, W3, b3, transport="ag"):
    x = np.asarray(x, np.float32)
    src = np.asarray(edge_index[0], dtype=np.int64)
    dst = np.asarray(edge_index[1], dtype=np.int64)

    has_bias = bool(np.any(b1) or np.any(b2) or np.any(b3))
    cfg = Cfg(has_bias, transport)

    deg = np.bincount(dst, minlength=N).astype(np.float32) + 1.0
    dinv = (1.0 / np.sqrt(deg)).astype(np.float32)

    perm, maxload = _balance_nodes(dst)
    assert maxload <= CPB_CAP * P, f"bin overflow: {maxload}"
    inv = np.empty(N, np.int64)
    inv[perm] = np.arange(N)
    ndst = inv[dst]

    order = np.argsort(ndst, kind="stable")
    src_s = src[order]
    ndst_s = ndst[order]

    CHB = np.asarray(CHUNK_BLOCKS) * BLK
    choff = np.concatenate([[0], np.cumsum(CHB)])
    rowoff = np.concatenate([[0], np.cumsum(CHB * NC)])
    # group row ranges in the table
    glo = [int(rowoff[g[0]]) for g in GROUPS]
    ghi = [int(rowoff[g[-1] + 1]) for g in GROUPS]
    NG = len(GROUPS)

    def remap_newid(node):
        c, l = node // SH, node % SH
        g = np.searchsorted(choff, l, side="right") - 1
        return rowoff[g] + c * CHB[g] + (l - choff[g])

    Wb_list = []
    for W, ow in ((W1, HID), (W2, HID), (W3, OUT)):
        w = np.asarray(W, np.float32).astype(ml_dtypes.bfloat16)
        Wb_list.append(np.ascontiguousarray(
            w.reshape(HID // P, P, ow).transpose(1, 0, 2).reshape(P, -1)))

    B3r = np.tile(np.asarray(b3, np.float32), (P, 1))
    B1r = np.tile(np.asarray(b1, np.float32), (P, 1))

    x_bf = x.astype(ml_dtypes.bfloat16)

    # ---- pass 1: per (core, block, group) edge slot data + counts ----
    # edge rows (remapped) per block, partitioned into groups
    block_data = [[None] * NT for _ in range(NC)]
    ncnt = np.zeros((NC, NT, NG), np.int64)
    for c in range(NC):
        for b in range(NT):
            blk_lo = c * SH + b * BLK
            i0 = np.searchsorted(ndst_s, blk_lo)
            i1 = np.searchsorted(ndst_s, blk_lo + BLK)
            bsrc = src_s[i0:i1]
            bdl = (ndst_s[i0:i1] - blk_lo).astype(np.int64)
            rows = remap_newid(inv[bsrc])
            bnorm = (dinv[bsrc] * dinv[perm[blk_lo + bdl]]).astype(np.float32)
            parts = []
            for gi in range(NG):
                m = (rows >= glo[gi]) & (rows < ghi[gi])
                parts.append((bsrc[m], bdl[m], bnorm[m], rows[m] - glo[gi]))
                ncnt[c, b, gi] = int(m.sum())
            block_data[c][b] = parts

    # uniform chunk counts per (block, group).  All slots carry a VALID
    # index (pads point at row 0, weighted zero by S): a skipped slot
    # would read stale SBUF, and 0 x NaN = NaN if a previous kernel
    # execution left non-finite bytes there.
    kk = np.zeros((NT, NG), np.int64)     # chunks per (block, group)
    for b in range(NT):
        for gi in range(NG):
            mx = int(ncnt[:, b, gi].max())
            kk[b, gi] = max(1, -(-mx // 128))
    vv = kk * 128                          # gathered count = all slots
    ktot = kk.sum(axis=1)                 # chunks per block
    soff = np.concatenate([[0], np.cumsum(ktot)])   # S chunk offset per block
    TOTCH = int(soff[-1])
    # idx free-dim offsets (units of 8 int16 cols per chunk)
    ioff = np.zeros((NT, NG), np.int64)
    run = 0
    for b in range(NT):
        for gi in range(NG):
            ioff[b, gi] = run
            run += kk[b, gi] * 8
    ITOT = int(run)

    cfg.kk, cfg.vv, cfg.ktot = kk, vv, ktot
    cfg.soff, cfg.ioff = soff, ioff
    cfg.TOTCH, cfg.ITOT = TOTCH, ITOT
    cfg.glo, cfg.ghi = glo, ghi
    cfg.perm = perm

    # ---- pass 2: build tables ----
    in_maps = []
    for c in range(NC):
        S_host = np.zeros((P, TOTCH, P), np.float32)
        idxs23 = np.zeros((P, ITOT), np.int16)
        xmsg = np.zeros((TOTCH * P, HID), ml_dtypes.bfloat16)
        dsqd = np.zeros((P, NT, P), np.float32)
        for b in range(NT):
            parts = block_data[c][b]
            kt = int(ktot[b])
            sl = np.zeros((kt * P, HID), ml_dtypes.bfloat16)   # slot messages
            slS = np.zeros((kt * P, P), np.float32)
            off = 0
            for gi in range(NG):
                bsrc, bdl, bnorm, rrel = parts[gi]
                n = len(bsrc)
                nk = int(kk[b, gi])
                nv = int(vv[b, gi])
                sl[off:off + n] = x_bf[bsrc]
                slS[off + np.arange(n), bdl] = bnorm
                I = np.zeros(nk * P, np.int64)
                I[:n] = rrel
                idxs23[:, ioff[b, gi]:ioff[b, gi] + nk * 8] = _wrap_idxs(I)
                off += nk * P
            # S: [p, chunk, j]
            S_host[:, soff[b]:soff[b + 1], :] = \
                slS.reshape(kt, P, P).transpose(1, 0, 2)
            # xmsg pre-wrapped: row p*kt + cjk = slot cjk*128+p
            xmsg[soff[b] * P:soff[b + 1] * P] = \
                sl.reshape(kt, P, HID).transpose(1, 0, 2).reshape(-1, HID)
            dd = dinv[perm[c * SH + b * BLK:c * SH + (b + 1) * BLK]] ** 2
            dsqd[np.arange(BLK), b, np.arange(BLK)] = dd
        xs_self = np.ascontiguousarray(x_bf[perm[c * SH:(c + 1) * SH]])
        in_maps.append({
            "xmsg": xmsg,
            "x_self": xs_self,
            "idxs23": idxs23,
            "S": S_host.astype(ml_dtypes.bfloat16).reshape(P, -1),
            "dsqd": dsqd.astype(ml_dtypes.bfloat16).reshape(P, -1),
            "W1b": Wb_list[0], "W2b": Wb_list[1], "W3b": Wb_list[2],
            "B1r": B1r, "B3r": B3r,
            "ident": np.eye(P, dtype=ml_dtypes.bfloat16),
        })
    return cfg, in_maps


def build(cfg: Cfg) -> bass.Bass:
    has_bias = cfg.has_bias
    kk, vv, ktot = cfg.kk, cfg.vv, cfg.ktot
    soff, ioff = cfg.soff, cfg.ioff
    TOTCH, ITOT = cfg.TOTCH, cfg.ITOT
    glo, ghi = cfg.glo, cfg.ghi
    NG = len(GROUPS)
    KGMAX = [int(kk[:, gi].max()) for gi in range(NG)]
    KTOTMAX = int(ktot.max())
    CHB = [g * BLK for g in CHUNK_BLOCKS]
    rowoff = np.concatenate([[0], np.cumsum(np.asarray(CHB) * NC)])

    nc = bacc.Bacc(None, target_bir_lowering=False, num_devices=NC,
                   num_swdge_queues=4)

    xmsg_in = nc.declare_dram_parameter("xmsg", [TOTCH * P, HID], BF16, isOutput=False)
    xs_in = nc.declare_dram_parameter("x_self", [SH, HID], BF16, isOutput=False)
    idxs23_in = nc.declare_dram_parameter("idxs23", [P, ITOT], I16, isOutput=False)
    S_in = nc.declare_dram_parameter("S", [P, TOTCH * P], BF16, isOutput=False)
    dsqd_in = nc.declare_dram_parameter("dsqd", [P, NT * P], BF16, isOutput=False)
    W1_in = nc.declare_dram_parameter("W1b", [P, KC * HID], BF16, isOutput=False)
    W2_in = nc.declare_dram_parameter("W2b", [P, KC * HID], BF16, isOutput=False)
    W3_in = nc.declare_dram_parameter("W3b", [P, KC * OUT], BF16, isOutput=False)
    B1_in = nc.declare_dram_parameter("B1r", [P, HID], F32, isOutput=False)
    B3_in = nc.declare_dram_parameter("B3r", [P, OUT], F32, isOutput=False)
    ident_in = nc.declare_dram_parameter("ident", [P, P], BF16, isOutput=False)
    out_ext = nc.declare_dram_parameter("out", [SH, OUT], F32, isOutput=True)

    a2a = cfg.transport == "a2a"
    if a2a:
        h1_loc = [nc.dram_tensor(f"h1rep{g}", [NC * CHB[g], HID], BF16)
                  for g in range(NCH)]
        z3_loc = [nc.dram_tensor(f"z3rep{g}", [NC * CHB[g], OUT], BF16)
                  for g in range(NCH)]
        h1_tab = nc.dram_tensor("h1tab", [N, HID], BF16)
        z3_tab = nc.dram_tensor("z3tab", [N, OUT], BF16)
    else:
        h1_loc = [nc.dram_tensor(f"h1loc{g}", [CHB[g], HID], BF16)
                  for g in range(NCH)]
        z3_loc = [nc.dram_tensor(f"z3loc{g}", [CHB[g], OUT], BF16)
                  for g in range(NCH)]
        h1_tab = nc.dram_tensor("h1tab", [N, HID], BF16, addr_space="Shared")
        z3_tab = nc.dram_tensor("z3tab", [N, OUT], BF16, addr_space="Shared")

    core_ids = list(range(NC))
    nc.gpsimd.bir_kernel_barrier_wait([core_ids])

    blk2chunk = []
    for g, nb in enumerate(CHUNK_BLOCKS):
        for j in range(nb):
            blk2chunk.append((g, j))

    with tile.TileContext(nc) as tc:
        with (
            tc.tile_pool(name="persist", bufs=1) as pp,
            tc.tile_pool(name="msga", bufs=2) as msga_pool,
            tc.tile_pool(name="msg", bufs=4) as msg_pool,
            tc.tile_pool(name="an", bufs=3) as an_pool,
            tc.tile_pool(name="xs", bufs=3) as xs_pool,
            tc.tile_pool(name="at", bufs=3) as at_pool,
            tc.tile_pool(name="ht", bufs=3) as ht_pool,
            tc.tile_pool(name="osb", bufs=3) as osb_pool,
            tc.tile_pool(name="psa", bufs=2, space="PSUM") as psa_pool,
            tc.tile_pool(name="pst", bufs=2, space="PSUM") as pst_pool,
            tc.tile_pool(name="psh", bufs=2, space="PSUM") as psh_pool,
            tc.tile_pool(name="psz", bufs=2, space="PSUM") as psz_pool,
        ):
            S_sb = pp.tile([P, TOTCH, P], BF16, tag="S")
            idxs23_sb = pp.tile([P, ITOT], I16, tag="idxs23")
            dsqd_sb = pp.tile([P, NT, P], BF16, tag="dsqd")
            W1_sb = pp.tile([P, KC, HID], BF16, tag="w1")
            W2_sb = pp.tile([P, KC, HID], BF16, tag="w2")
            W3_sb = pp.tile([P, KC, OUT], BF16, tag="w3")
            B1_sb = pp.tile([P, HID], F32, tag="b1")
            B3_sb = pp.tile([P, OUT], F32, tag="b3")
            ident_sb = pp.tile([P, P], BF16, tag="ident")
            hself = pp.tile([P, NT, HID], BF16, tag="hself")
            zself = pp.tile([P, NT, OUT], BF16, tag="zself")

            # weights/ident/idxs/S on the scalar HWDGE ring
            nc.scalar.dma_start(out=idxs23_sb[:], in_=idxs23_in[:])
            nc.scalar.dma_start(out=W1_sb[:], in_=W1_in[:].rearrange("p (c o) -> p c o", c=KC))
            nc.scalar.dma_start(out=W2_sb[:], in_=W2_in[:].rearrange("p (c o) -> p c o", c=KC))
            nc.scalar.dma_start(out=W3_sb[:], in_=W3_in[:].rearrange("p (c o) -> p c o", c=KC))
            nc.scalar.dma_start(out=ident_sb[:], in_=ident_in[:])
            nc.scalar.dma_start(out=dsqd_sb[:], in_=dsqd_in[:].rearrange("p (b j) -> p b j", b=NT))
            if has_bias:
                nc.scalar.dma_start(out=B1_sb[:], in_=B1_in[:])
                nc.scalar.dma_start(out=B3_sb[:], in_=B3_in[:])
            # stream S in per-2-block pieces so block 0 starts early
            for b0 in range(0, NT, 2):
                c0 = int(soff[b0]) * P
                c1 = int(soff[min(b0 + 2, NT)]) * P
                nc.scalar.dma_start(
                    out=S_sb[:].rearrange("p k j -> p (k j)")[:, c0:c1],
                    in_=S_in[:, c0:c1])

            regs = {}
            for v in sorted(set(int(x) for x in vv.ravel())):
                regs[v] = nc.gpsimd.to_reg(v)

            def distribute(loc, tab, g):
                with tc.high_priority():
                    if a2a:
                        for j in range(1, NC):
                            nc.sync.dma_start(
                                out=loc[g][j * CHB[g]:(j + 1) * CHB[g], :],
                                in_=loc[g][0:CHB[g], :])
                        nc.gpsimd.collective_compute(
                            "AllToAll", mybir.AluOpType.bypass,
                            ins=[loc[g][:].opt()],
                            outs=[tab[rowoff[g]:rowoff[g + 1], :].opt()],
                            replica_groups=[core_ids])
                    else:
                        nc.gpsimd.collective_compute(
                            "AllGather", mybir.AluOpType.bypass,
                            ins=[loc[g][:].opt()],
                            outs=[tab[rowoff[g]:rowoff[g + 1], :].opt()],
                            replica_groups=[core_ids])

            qrot = [0]

            def gather(tab, b, gi, width):
                """Sub-gather for block b, chunk-group gi."""
                nk = int(kk[b, gi])
                msg = msg_pool.tile([P, KGMAX[gi], width], BF16,
                                    tag=f"msg{width}_{gi}")
                q = qrot[0] % 4
                qrot[0] += 1
                nc.gpsimd.dma_gather(
                    out_ap=msg[:, :nk, :], in_ap=tab[glo[gi]:ghi[gi], :],
                    idxs_ap=idxs23_sb[:, ioff[b, gi]:ioff[b, gi] + nk * 8],
                    num_idxs=nk * P, num_idxs_reg=regs[int(vv[b, gi])],
                    elem_size=width, single_packet=False,
                    queue_num=q)
                return msg

            def agg(b, msgs, self_sb, width, pool, tag):
                """PSUM agg over all sub-group chunks + diag self term."""
                pA = pool.tile([P, width], F32, tag=tag)
                first = True
                for gi in range(NG):
                    nk = int(kk[b, gi])
                    base = int(soff[b] + sum(int(kk[b, g2]) for g2 in range(gi)))
                    for k in range(nk):
                        nc.tensor.matmul(
                            out=pA[:BLK, :],
                            lhsT=S_sb[:, base + k, :BLK],
                            rhs=msgs[gi][:, k, :width],
                            start=first, stop=False)
                        first = False
                nc.tensor.matmul(
                    out=pA[:BLK, :],
                    lhsT=dsqd_sb[:BLK, b, :BLK],
                    rhs=self_sb,
                    start=False, stop=True)
                return pA

            def aT_from(pA):
                a_node = an_pool.tile([P, HID], BF16, tag="an")
                nc.scalar.activation(
                    out=a_node[:BLK, :], in_=pA[:BLK, :],
                    func=mybir.ActivationFunctionType.Copy)
                aT = at_pool.tile([P, KC, BLK], BF16, tag="at")
                for fh in range(KC):
                    pT = pst_pool.tile([P, P], BF16, tag="pt")
                    nc.tensor.transpose(
                        out=pT[:, :BLK],
                        in_=a_node[:BLK, fh * P:(fh + 1) * P],
                        identity=ident_sb[:BLK, :BLK])
                    nc.scalar.activation(
                        out=aT[:, fh, :], in_=pT[:, :BLK],
                        func=mybir.ActivationFunctionType.Copy)
                return aT

            # =========== phase A: layer 1 ===========
            for b in range(NT):
                g, brel = blk2chunk[b]
                kt = int(ktot[b])
                msg = msga_pool.tile([P, KTOTMAX, HID], BF16, tag="msgA")
                nc.sync.dma_start(
                    out=msg[:, :kt, :],
                    in_=xmsg_in[int(soff[b]) * P:int(soff[b + 1]) * P, :].rearrange(
                        "(p c) f -> p c f", p=P))
                xs = xs_pool.tile([P, HID], BF16, tag="xs")
                nc.sync.dma_start(out=xs[:BLK, :],
                                  in_=xs_in[b * BLK:(b + 1) * BLK, :])
                # aggregate all kt chunks from the single stream
                pA = psa_pool.tile([P, HID], F32, tag="pa")
                for k in range(kt):
                    nc.tensor.matmul(
                        out=pA[:BLK, :],
                        lhsT=S_sb[:, int(soff[b]) + k, :BLK],
                        rhs=msg[:, k, :],
                        start=(k == 0), stop=False)
                nc.tensor.matmul(
                    out=pA[:BLK, :],
                    lhsT=dsqd_sb[:BLK, b, :BLK],
                    rhs=xs[:BLK, :],
                    start=False, stop=True)
                aT = aT_from(pA)
                pH = psh_pool.tile([P, HID], F32, tag="ph")
                for kc in range(KC):
                    nc.tensor.matmul(
                        out=pH[:BLK, :], lhsT=aT[:, kc, :],
                        rhs=W1_sb[:, kc, :],
                        start=(kc == 0), stop=(kc == KC - 1))
                if has_bias:
                    nc.vector.tensor_add(out=pH[:BLK, :], in0=pH[:BLK, :],
                                         in1=B1_sb[:BLK, :])
                nc.scalar.activation(
                    out=hself[:BLK, b, :], in_=pH[:BLK, :],
                    func=mybir.ActivationFunctionType.Relu)
                nc.sync.dma_start(out=h1_loc[g][brel * BLK:(brel + 1) * BLK, :],
                                  in_=hself[:BLK, b, :])
                if brel == CHUNK_BLOCKS[g] - 1:
                    distribute(h1_loc, h1_tab, g)

            # =========== phase B: layer 2 + z3 ===========
            for b in range(NT):
                g, brel = blk2chunk[b]
                msgs = [gather(h1_tab, b, gi, HID) for gi in range(NG)]
                pA = agg(b, msgs, hself[:BLK, b, :], HID, psa_pool, "pa")
                aT = aT_from(pA)
                hT = ht_pool.tile([P, KC, BLK], BF16, tag="ht")
                for fo in range(KC):
                    pT = pst_pool.tile([P, P], F32, tag="pt")
                    for kc in range(KC):
                        nc.tensor.matmul(
                            out=pT[:, :BLK],
                            lhsT=W2_sb[:, kc, fo * P:(fo + 1) * P],
                            rhs=aT[:, kc, :],
                            start=(kc == 0), stop=(kc == KC - 1))
                    nc.scalar.activation(
                        out=hT[:, fo, :], in_=pT[:, :BLK],
                        func=mybir.ActivationFunctionType.Relu)
                pz = psz_pool.tile([P, OUT], F32, tag="pz")
                for kc in range(KC):
                    nc.tensor.matmul(
                        out=pz[:BLK, :], lhsT=hT[:, kc, :],
                        rhs=W3_sb[:, kc, :],
                        start=(kc == 0), stop=(kc == KC - 1))
                nc.scalar.activation(
                    out=zself[:BLK, b, :], in_=pz[:BLK, :],
                    func=mybir.ActivationFunctionType.Copy)
                nc.sync.dma_start(out=z3_loc[g][brel * BLK:(brel + 1) * BLK, :],
                                  in_=zself[:BLK, b, :])
                if brel == CHUNK_BLOCKS[g] - 1:
                    distribute(z3_loc, z3_tab, g)

            # =========== phase C: layer 3 ===========
            for b in range(NT):
                msgs = [gather(z3_tab, b, gi, OUT) for gi in range(NG)]
                pO = agg(b, msgs, zself[:BLK, b, :], OUT, psz_pool, "pz")
                o_sb = osb_pool.tile([P, OUT], F32, tag="osb")
                nc.scalar.activation(
                    out=o_sb[:BLK, :], in_=pO[:BLK, :],
                    func=mybir.ActivationFunctionType.Copy)
                if has_bias:
                    nc.vector.tensor_add(out=o_sb[:BLK, :], in0=o_sb[:BLK, :],
                                         in1=B3_sb[:BLK, :])
                nc.sync.dma_start(out=out_ext[b * BLK:(b + 1) * BLK, :],
                                  in_=o_sb[:BLK, :])

    nc.finalize()
    split_sync_waits(nc)
    return nc


_counter = [0]


def split_sync_waits(nc, maxw=1):
    n_split = 0
    for f in nc.m.functions:
        for bb in f.blocks:
            insts = list(bb.instructions)
            out = []
            changed = False
            for inst in insts:
                si = inst.sync_info
                if si is not None and len(si.on_wait) > maxw:
                    waits = list(si.on_wait)
                    keep = waits[-maxw:] if maxw else []
                    rest = waits[: len(waits) - maxw]
                    for w in rest:
                        _counter[0] += 1
                        nop = mybir.InstNoOp(
                            name=f"wspill-{_counter[0]}",
                            engine=inst.engine,
                            bass_nofuse=True,
                            sync_info=mybir.SyncInfo(on_wait=[w], on_update=[]),
                        )
                        nc.register_instruction(nop)
                        out.append(nop)
                    si.on_wait = keep
                    changed = True
                    n_split += 1
                out.append(inst)
            if changed:
                bb.instructions = out
    return n_split


def kernel(**inputs):
    from concourse.bass_utils import run_bass_kernel_spmd

    x = np.asarray(inputs["x"], np.float32)
    edge_index = np.asarray(inputs["edge_index"])
    cfg, in_maps = prep(
        x, edge_index,
        np.asarray(inputs["W1"], np.float32), np.asarray(inputs["b1"], np.float32),
        np.asarray(inputs["W2"], np.float32), np.asarray(inputs["b2"], np.float32),
        np.asarray(inputs["W3"], np.float32), np.asarray(inputs["b3"], np.float32))
    nc = build(cfg)
    res = run_bass_kernel_spmd(nc, in_maps, core_ids=list(range(NC)))
    out_perm = np.concatenate([res.results[c]["out"] for c in range(NC)], axis=0)
    out = np.empty_like(out_perm)
    out[cfg.perm] = out_perm
    return out.astype(np.float32)
